# revision 18
# baseline (speedup 1.0000x reference)
"""Child-Sum TreeLSTM (perfect binary tree, depth 14) on 8 Trainium2 NeuronCores.

Strategy (v2)
-------------
Heap-order contiguous node sharding: core k owns nodes [k*n/8, (k+1)*n/8) of
every level lvl >= 3 (n = 2^lvl). Children of a core's node range at level lvl
are exactly its node range at level lvl+1, so levels 13..3 run with zero
cross-core communication. The top 7 nodes (levels 2..0) run in the SAME
program: after level 3 an AllGather (DRAM bounce) distributes the 8 level-3
(h, c) states to every core, which then computes levels 2..0 redundantly;
core 0's root output is used.

Within a core, state is transposed: [mem_dim(1024) partitions x n nodes], one
SBUF tile [128, 8*n] per level (M-tile m of the mem dim = col block m). Per
level, gate pre-activations accumulate in PSUM from 8 K-chunk matmuls against
Wh; the precomputed x-projection (bias baked in via an extra contraction row
of the x-GEMM) is then added into PSUM by the vector engine (no identity
matmuls). All matmul inputs bf16 (fp32 PSUM accumulation); stored states bf16.
x-projections for all nodes are computed upfront in one dense pass (leaf level
fused into it): level 12 staged in DRAM, levels 11..3 and the top-7 nodes kept
resident in SBUF.
"""

import numpy as np
import ml_dtypes
from contextlib import ExitStack

import concourse.bass as bass
import concourse.tile as tile
from concourse import bacc, mybir
from concourse.bass_utils import run_bass_kernel_spmd

BF16 = ml_dtypes.bfloat16
P = 128
MEM = 1024
IN = 300
DEPTH = 14
NCORE = 8
MT = MEM // P  # 8 M-tiles of the mem dim

NX = 2047  # per-core nodes, levels 13..3
NTOP = 7  # top nodes (levels 2..0)
NXT = NX + NTOP  # 2054
NRES = 511  # resident-level nodes (levels 11..3), kept SBUF-resident
RES0 = 1536  # first resident-level node col (level 11)

GATES = "ifou"
SL = {"i": 0, "o": 1, "u": 2, "f": 3}  # per-m block order in xres/xtop/xproj
SIG = mybir.ActivationFunctionType.Sigmoid
TANH = mybir.ActivationFunctionType.Tanh
IDENT = mybir.ActivationFunctionType.Identity
ACT_FN = {"i": SIG, "f": SIG, "o": SIG, "u": TANH}

# per-core column offset of level lvl within the node axis (levels 13..3)
OFF = {13: 0}
for _lvl in range(12, 2, -1):
    OFF[_lvl] = OFF[_lvl + 1] + (2 ** (_lvl + 1)) // NCORE

LEAF_CHUNKS = [(0, 512), (512, 512)]  # lvl 13, fused, i/o/u only
L12_CHUNK = (1024, 512)  # lvl 12 -> DRAM
RES_CHUNK = (RES0, NRES)  # lvl 11..3 -> resident SBUF
TOP_CHUNK = (NX, NTOP)  # lvl 2..0 -> resident SBUF

F32 = mybir.dt.float32
BF = mybir.dt.bfloat16


def _emit_xproj_and_leaf(nc, xt, wxT_d, xproj, xres, xtop_sb, pools):
    """x-projection pass (bias baked in as an extra contraction row) with leaf
    level (13) fused. Writes the L12 chunk to DRAM `xproj`, resident chunk to
    SBUF `xres` ((m,sl)-major: col (m*4+sl)*NRES + ...), top chunk to
    `xtop_sb` ((m*4+sl)*NTOP). Returns leaf state tiles h13, c13."""
    xpp, gp, hp, psum = pools
    h13 = hp.tile([P, MT * 1024], BF, tag="h_odd", name="h13")
    c13 = hp.tile([P, MT * 1024], BF, tag="c_odd", name="c13")
    wx3 = wxT_d.rearrange("p (b c) -> p b c", c=MEM)
    for m in range(MT):
        leaf = {}
        for gA, gB in (("i", "o"), ("u", "f")):
            wxs = {}
            for g in (gA, gB):
                gi = GATES.index(g)
                wx = gp.tile([P, 3 * P], BF, tag="wx", bufs=4, name=f"wx_{m}_{g}")
                nc.sync.dma_start(
                    wx[:].rearrange("p (b c) -> p b c", c=P),
                    wx3[:, gi * 3 : gi * 3 + 3, m * P : (m + 1) * P],
                )
                wxs[g] = wx
            for c0, ncol in LEAF_CHUNKS + [L12_CHUNK, RES_CHUNK, TOP_CHUNK]:
                pss = {}
                for g in (gA, gB):
                    if (c0, ncol) in LEAF_CHUNKS and g == "f":
                        continue
                    ps = psum.tile([P, ncol], F32, tag="ps", bufs=8, name=f"ps_{m}_{g}{c0}")
                    pss[g] = ps
                    for kx in range(2):
                        nc.tensor.matmul(
                            ps[:],
                            wxs[g][:, kx * P : (kx + 1) * P],
                            xt[:, kx * NXT + c0 : kx * NXT + c0 + ncol],
                            start=(kx == 0),
                            stop=False,
                        )
                # kx=2 chunk has only 45 contraction rows; the two gates' copies
                # live at partition bases 0 / 64 so these two MMs run in
                # disjoint PE row-groups, i.e. concurrently.
                for g in pss:
                    base = 0 if g in "iu" else 64
                    nc.tensor.matmul(
                        pss[g][:],
                        wxs[g][base : base + 45, 2 * P : 3 * P],
                        xt[base : base + 45, 2 * NXT + c0 : 2 * NXT + c0 + ncol],
                        start=False,
                        stop=True,
                        tile_position=(base, 0),
                    )
                for g, ps in pss.items():
                    sl = SL[g]
                    if (c0, ncol) in LEAF_CHUNKS:
                        tg = gp.tile([P, ncol], F32, tag="g", bufs=8, name=f"lf_{m}_{g}{c0}")
                        nc.scalar.activation(tg[:], ps[:], ACT_FN[g])
                        leaf[(g, c0)] = tg
                    elif (c0, ncol) == RES_CHUNK:
                        nc.vector.tensor_copy(
                            xres[:, (m * 4 + sl) * NRES : (m * 4 + sl + 1) * NRES], ps[:]
                        )
                    elif (c0, ncol) == TOP_CHUNK:
                        nc.vector.tensor_copy(
                            xtop_sb[:, (m * 4 + sl) * NTOP : (m * 4 + sl + 1) * NTOP], ps[:]
                        )
                    else:  # L12 chunk
                        xp = xpp.tile([P, ncol], BF, tag="xp", bufs=6, name=f"xp_{m}_{g}{c0}")
                        nc.scalar.activation(xp[:], ps[:], IDENT)
                        nc.sync.dma_start(
                            xproj[:, (m * 4 + sl) * 512 : (m * 4 + sl) * 512 + ncol], xp[:]
                        )
        # leaf combine: c = i*u ; h = o*tanh(c)
        for c0, ncol in LEAF_CHUNKS:
            cs = c13[:, m * 1024 + c0 : m * 1024 + c0 + ncol]
            nc.vector.tensor_mul(cs, leaf[("i", c0)][:], leaf[("u", c0)][:])
            th = gp.tile([P, ncol], F32, tag="g", bufs=8, name=f"th13_{m}_{c0}")
            nc.scalar.activation(th[:], cs, TANH)
            nc.vector.tensor_mul(h13[:, m * 1024 + c0 : m * 1024 + c0 + ncol], leaf[("o", c0)][:], th[:])
    return h13, c13


def _emit_level(nc, wh, h_ch, c_ch, n, xp_iou, xp_f, par, pools, lvl):
    """One non-leaf level. h_ch/c_ch: single tiles [128, 8*2n] bf16.
    xp_iou(m) -> AP [128, 3n] (or [128,3,n]) with xproj+bias for i|o|u.
    xp_f(m) -> AP [128, n]. Returns (h_new, c_new) tiles [128, 8*n] bf16."""
    xpp, gp, hp, psum = pools
    n2 = 2 * n
    combined = 3 * n <= 512
    fused_f = n2 <= 512
    fgi = GATES.index("f")

    def wh_ap(gi, k, m):
        return wh[:, (gi * MT + k) * MEM + m * P : (gi * MT + k) * MEM + (m + 1) * P]

    h_new = hp.tile([P, MT * n], BF, tag=f"h_{'odd' if par else 'even'}", name=f"h_{lvl}")
    c_new = hp.tile([P, MT * n], BF, tag=f"c_{'odd' if par else 'even'}", name=f"c_{lvl}")

    # f-gate matmuls first: they depend only on child h, so the PE has dense
    # work at level start while the DVE computes hsum.
    ff_of = {}
    if fused_f:
        for m in range(MT):
            psf = psum.tile([P, n2], F32, tag="ps", bufs=8, name=f"psf_{lvl}_{m}")
            for k in range(MT):
                nc.tensor.matmul(
                    psf[:], wh_ap(fgi, k, m), h_ch[:, k * n2 : (k + 1) * n2],
                    start=(k == 0), stop=(k == MT - 1),
                )
            xpf = xp_f(m)
            tmpf = gp.tile([P, n2], F32, tag="g", bufs=8, name=f"tmpf_{lvl}_{m}")
            nc.vector.tensor_add(tmpf[:, 0:n2:2], psf[:, 0:n2:2], xpf)
            nc.vector.tensor_add(tmpf[:, 1:n2:2], psf[:, 1:n2:2], xpf)
            ff = gp.tile([P, n2], BF, tag="ff", bufs=8, name=f"ff_{lvl}_{m}")
            nc.scalar.activation(ff[:], tmpf[:], SIG)
            ff_of[m] = ff

    hs = hp.tile([P, MT * n], BF, tag=f"hs_{par}", name=f"hs_{lvl}")
    for k in range(MT):
        nc.vector.tensor_add(
            hs[:, k * n : (k + 1) * n], h_ch[:, k * n2 : (k + 1) * n2 : 2], h_ch[:, k * n2 + 1 : (k + 1) * n2 : 2]
        )

    for m in range(MT):
        gio = {}
        if combined:
            ps3 = psum.tile([P, 3 * n], F32, tag="ps", bufs=8, name=f"ps3_{lvl}_{m}")
            # NOTE: start=True clears has_written for the WHOLE bank, so only
            # the very first matmul in this psum tile may set it; the other
            # slices' first matmuls overwrite-where-unset (start=False).
            for k in range(MT):
                hsk = hs[:, k * n : (k + 1) * n]
                for sl, g in enumerate("iou"):
                    nc.tensor.matmul(
                        ps3[:, sl * n : (sl + 1) * n], wh_ap(GATES.index(g), k, m), hsk,
                        start=(k == 0 and sl == 0), stop=(k == MT - 1 and sl == 2),
                        skip_group_check=True,
                    )
            pre3 = gp.tile([P, 3 * n], F32, tag="g", bufs=8, name=f"pre3_{lvl}_{m}")
            nc.vector.tensor_add(
                pre3[:].rearrange("p (s c) -> p s c", c=n),
                ps3[:].rearrange("p (s c) -> p s c", c=n),
                xp_iou(m),
            )
            gt = gp.tile([P, 3 * n], F32, tag="g", bufs=8, name=f"gt_{lvl}_{m}")
            nc.scalar.activation(gt[:, 0 : 2 * n], pre3[:, 0 : 2 * n], SIG)
            nc.scalar.activation(gt[:, 2 * n : 3 * n], pre3[:, 2 * n : 3 * n], TANH)
            gio["i"], gio["o"], gio["u"] = gt[:, 0:n], gt[:, n : 2 * n], gt[:, 2 * n : 3 * n]
        else:
            xpm = xp_iou(m)
            for sl, g in enumerate("iou"):
                ps = psum.tile([P, n], F32, tag="ps", bufs=8, name=f"ps_{lvl}_{m}{g}")
                for k in range(MT):
                    nc.tensor.matmul(
                        ps[:], wh_ap(GATES.index(g), k, m), hs[:, k * n : (k + 1) * n],
                        start=(k == 0), stop=(k == MT - 1),
                    )
                pre = gp.tile([P, n], F32, tag="g", bufs=8, name=f"pre_{lvl}_{m}{g}")
                nc.vector.tensor_add(pre[:], ps[:], xpm[:, sl, :] if len(xpm.shape) == 3 else xpm[:, sl * n : (sl + 1) * n])
                tg = gp.tile([P, n], F32, tag="g", bufs=8, name=f"t{g}_{lvl}_{m}")
                nc.scalar.activation(tg[:], pre[:], ACT_FN[g])
                gio[g] = tg[:]

        if fused_f:
            prod = gp.tile([P, n2], F32, tag="g", bufs=8, name=f"prod_{lvl}_{m}")
            nc.vector.tensor_mul(prod[:], ff_of[m][:], c_ch[:, m * n2 : (m + 1) * n2])
            fc = gp.tile([P, n], F32, tag="g", bufs=8, name=f"fc_{lvl}_{m}")
            nc.vector.tensor_add(fc[:], prod[:, 0:n2:2], prod[:, 1:n2:2])
        else:
            psL = psum.tile([P, n], F32, tag="ps", bufs=8, name=f"psL_{lvl}_{m}")
            psR = psum.tile([P, n], F32, tag="ps", bufs=8, name=f"psR_{lvl}_{m}")
            for k in range(MT):
                w = wh_ap(fgi, k, m)
                nc.tensor.matmul(psL[:], w, h_ch[:, k * n2 : (k + 1) * n2 : 2], start=(k == 0), stop=(k == MT - 1))
                nc.tensor.matmul(psR[:], w, h_ch[:, k * n2 + 1 : (k + 1) * n2 : 2], start=(k == 0), stop=(k == MT - 1))
            xpf = xp_f(m)
            preL = gp.tile([P, n], F32, tag="g", bufs=8, name=f"preL_{lvl}_{m}")
            nc.vector.tensor_add(preL[:], psL[:], xpf)
            preR = gp.tile([P, n], F32, tag="g", bufs=8, name=f"preR_{lvl}_{m}")
            nc.vector.tensor_add(preR[:], psR[:], xpf)
            fL = gp.tile([P, n], F32, tag="g", bufs=8, name=f"fL_{lvl}_{m}")
            nc.scalar.activation(fL[:], preL[:], SIG)
            fR = gp.tile([P, n], F32, tag="g", bufs=8, name=f"fR_{lvl}_{m}")
            nc.scalar.activation(fR[:], preR[:], SIG)
            t1 = gp.tile([P, n], F32, tag="g", bufs=8, name=f"t1_{lvl}_{m}")
            nc.vector.tensor_mul(t1[:], fL[:], c_ch[:, m * n2 : (m + 1) * n2 : 2])
            t2 = gp.tile([P, n], F32, tag="g", bufs=8, name=f"t2_{lvl}_{m}")
            nc.vector.tensor_mul(t2[:], fR[:], c_ch[:, m * n2 + 1 : (m + 1) * n2 : 2])
            fc = gp.tile([P, n], F32, tag="g", bufs=8, name=f"fc_{lvl}_{m}")
            nc.vector.tensor_add(fc[:], t1[:], t2[:])

        tiu = gp.tile([P, n], F32, tag="g", bufs=8, name=f"tiu_{lvl}_{m}")
        nc.vector.tensor_mul(tiu[:], gio["i"], gio["u"])
        cm = c_new[:, m * n : (m + 1) * n]
        nc.vector.tensor_add(cm, tiu[:], fc[:])
        th = gp.tile([P, n], F32, tag="g", bufs=8, name=f"th_{lvl}_{m}")
        nc.scalar.activation(th[:], cm, TANH)
        nc.vector.tensor_mul(h_new[:, m * n : (m + 1) * n], gio["o"], th[:])
    return h_new, c_new


def build_kernel():
    nc = bacc.Bacc("TRN2", target_bir_lowering=False, debug=False, num_devices=NCORE)
    xT_d = nc.dram_tensor("xT", [P, 3 * NXT], BF, kind="ExternalInput").ap()
    wxT_d = nc.dram_tensor("wxT", [P, 4 * 3 * MEM], BF, kind="ExternalInput").ap()
    whT_d = nc.dram_tensor("whT", [P, 4 * MT * MEM], BF, kind="ExternalInput").ap()
    root_d = nc.dram_tensor("root", [P, 16], F32, kind="ExternalOutput").ap()
    dbg_pre_d = nc.dram_tensor("dbg_pre", [P, 16], F32, kind="ExternalOutput").ap()
    dbg_ag_d = nc.dram_tensor("dbg_ag", [P, 128], F32, kind="ExternalOutput").ap()

    with tile.TileContext(nc) as tc, ExitStack() as ctx:
        const = ctx.enter_context(tc.tile_pool(name="const", bufs=1))
        xpp = ctx.enter_context(tc.tile_pool(name="xpp", bufs=8))
        gp = ctx.enter_context(tc.tile_pool(name="gp", bufs=8))
        hp = ctx.enter_context(tc.tile_pool(name="hp", bufs=1))
        psum = ctx.enter_context(tc.tile_pool(name="psum", bufs=8, space="PSUM"))
        dram = ctx.enter_context(tc.tile_pool(name="dram", bufs=1, space="DRAM"))
        pools = (xpp, gp, hp, psum)

        xt = const.tile([P, 3 * NXT], BF, name="xt")
        for kx in range(3):
            nc.sync.dma_start(xt[:, kx * NXT : (kx + 1) * NXT], xT_d[:, kx * NXT : (kx + 1) * NXT])
        # weights for the h-GEMMs load during the xproj/leaf pass
        wh = const.tile([P, 4 * MT * MEM], BF, name="wh_sb")
        for gi in range(4):
            s = gi * MT * MEM
            nc.sync.dma_start(wh[:, s : s + MT * MEM], whT_d[:, s : s + MT * MEM])
        xres = const.tile([P, 32 * NRES], BF, name="xres")
        xtop_sb = const.tile([P, 32 * NTOP], BF, name="xtop_sb")
        xproj = dram.tile([P, 32 * 512], BF, name="xproj")  # level 12

        h_ch, c_ch = _emit_xproj_and_leaf(nc, xt, wxT_d, xproj, xres, xtop_sb, pools)

        for lvl in range(12, 2, -1):
            n = 1 << (lvl - 3)
            par = lvl & 1
            if lvl == 12:

                def xp_iou(m, n=n):
                    xp = xpp.tile([P, 3 * n], BF, tag="xp12", bufs=2, name=f"xpl12_{m}")
                    nc.sync.dma_start(xp[:], xproj[:, (m * 4) * 512 : (m * 4) * 512 + 3 * n])
                    return xp[:].rearrange("p (s c) -> p s c", c=n)

                def xp_f(m, n=n):
                    xp = xpp.tile([P, n], BF, tag="xp", bufs=6, name=f"xpf12_{m}")
                    nc.sync.dma_start(xp[:], xproj[:, (m * 4 + 3) * 512 : (m * 4 + 3) * 512 + n])
                    return xp[:]

            else:
                off = OFF[lvl] - RES0

                def xp_iou(m, off=off, n=n):
                    v = xres[:].rearrange("p (m s c) -> p m s c", s=4, c=NRES)
                    return v[:, m, 0:3, off : off + n]

                def xp_f(m, off=off, n=n):
                    return xres[:, (m * 4 + 3) * NRES + off : (m * 4 + 3) * NRES + off + n]

            h_ch, c_ch = _emit_level(nc, wh, h_ch, c_ch, n, xp_iou, xp_f, par, pools, lvl)

        # ---- AllGather the 8 level-3 (h, c) states; all cores compute top 7 nodes ----
        agin_sb = gp.tile([P, 16], F32, tag="g", bufs=8, name="agin_sb")
        nc.vector.tensor_copy(agin_sb[:, 0:8], h_ch[:])
        nc.vector.tensor_copy(agin_sb[:, 8:16], c_ch[:])
        ag_in = dram.tile([P, 16], F32, name="ag_in")
        ag_out = dram.tile([NCORE, 16 * P], F32, name="ag_out")  # dim0 = source core j
        nc.gpsimd.dma_start(ag_in[:], agin_sb[:])
        nc.gpsimd.collective_compute(
            "AllGather",
            mybir.AluOpType.bypass,
            replica_groups=[list(range(NCORE))],
            ins=[ag_in.opt()],
            outs=[ag_out.opt()],
        )
        # AllGather concatenates the FLAT per-core buffers; core j's [P,16]
        # payload is ag_out row j, flat p*16+k. Pull each back as [128,16].
        ag_sb = gp.tile([P, 128], F32, tag="g", bufs=8, name="ag_sb")
        for j in range(NCORE):
            src = ag_out[j : j + 1, :].rearrange("o (p k) -> (o p) k", k=16)
            nc.gpsimd.dma_start(ag_sb[:, j * 16 : (j + 1) * 16], src)
        nc.sync.dma_start(dbg_pre_d[:], agin_sb[:])
        nc.sync.dma_start(dbg_ag_d[:], ag_sb[:])
        # unpack: h_ch[:, k*8 + j] = ag_sb[:, j*16 + k]  (j = core, k = m-tile)
        h_ch = hp.tile([P, MT * 8], BF, tag="h_odd", name="h3_full")
        c_ch = hp.tile([P, MT * 8], BF, tag="c_odd", name="c3_full")
        agv = ag_sb[:].rearrange("p (j k) -> p k j", k=16)
        nc.vector.tensor_copy(h_ch[:].rearrange("p (k j) -> p k j", j=8), agv[:, 0:8, :])
        nc.vector.tensor_copy(c_ch[:].rearrange("p (k j) -> p k j", j=8), agv[:, 8:16, :])

        TOFF = {2: 0, 1: 4, 0: 6}
        for lvl in range(2, -1, -1):
            n = 1 << lvl
            toff = TOFF[lvl]
            par = lvl & 1

            def xp_iou(m, toff=toff, n=n):
                v = xtop_sb[:].rearrange("p (m s c) -> p m s c", s=4, c=NTOP)
                return v[:, m, 0:3, toff : toff + n]

            def xp_f(m, toff=toff, n=n):
                return xtop_sb[:, (m * 4 + 3) * NTOP + toff : (m * 4 + 3) * NTOP + toff + n]

            h_ch, c_ch = _emit_level(nc, wh, h_ch, c_ch, n, xp_iou, xp_f, par, pools, lvl)

        out32 = gp.tile([P, 16], F32, tag="g", bufs=8, name="out32")
        nc.vector.tensor_copy(out32[:, 0:8], c_ch[:])
        nc.vector.tensor_copy(out32[:, 8:16], h_ch[:])
        nc.sync.dma_start(root_d[:], out32[:])
    nc.compile()
    return nc


_CACHE = {}


def _get_programs():
    if "a" not in _CACHE:
        _CACHE["a"] = build_kernel()
    return _CACHE["a"]


def _prep_host_inputs(embs, Ws, bs):
    wxT = np.zeros((P, 4 * 3 * MEM), BF16)
    whT = np.zeros((P, 4 * MT * MEM), BF16)
    for gi, g in enumerate(GATES):
        WxT = Ws[g + "x"].T.astype(BF16)  # [300, 1024]
        for kx in range(3):
            rows = WxT[kx * P : (kx + 1) * P]
            wxT[: rows.shape[0], (gi * 3 + kx) * MEM : (gi * 3 + kx + 1) * MEM] = rows
        # bias baked in as an extra contraction row of chunk kx=2 (44 x rows +
        # ones row). Gates o/f live at partition base 64 so their kx=2 MMs can
        # row-group-pack against i/u's at base 0.
        base = 0 if g in "iu" else 64
        if base:
            blk = wxT[base : base + 44, (gi * 3 + 2) * MEM : (gi * 3 + 3) * MEM]
            blk[:] = wxT[0:44, (gi * 3 + 2) * MEM : (gi * 3 + 3) * MEM]
            wxT[0:44, (gi * 3 + 2) * MEM : (gi * 3 + 3) * MEM] = 0
        wxT[base + 44, (gi * 3 + 2) * MEM : (gi * 3 + 3) * MEM] = bs[g].astype(BF16)
        WhT = Ws[g + "h"].T.astype(BF16)  # [1024, 1024]
        for k in range(MT):
            whT[:, (gi * MT + k) * MEM : (gi * MT + k + 1) * MEM] = WhT[k * P : (k + 1) * P]

    x_top = embs[[3, 4, 5, 6, 1, 2, 0]].T  # [300, 7], heap order per level
    in_maps = []
    for k in range(NCORE):
        cols = []
        for lvl in range(DEPTH - 1, 2, -1):
            n = 1 << lvl
            nl = n // NCORE
            cols.append(embs[n - 1 + k * nl : n - 1 + (k + 1) * nl].T)
        x_all = np.concatenate(cols + [x_top], axis=1)  # [300, 2054]
        xT = np.zeros((P, 3 * NXT), BF16)
        for kx in range(3):
            rows = x_all[kx * P : (kx + 1) * P].astype(BF16)
            xT[: rows.shape[0], kx * NXT : (kx + 1) * NXT] = rows
        # chunk-2 x rows duplicated at partition base 64 (for o/f row-group
        # packing) plus the ones rows pairing the bias rows
        xT[64:108, 2 * NXT : 3 * NXT] = xT[0:44, 2 * NXT : 3 * NXT]
        xT[44, 2 * NXT : 3 * NXT] = BF16(1.0)
        xT[108, 2 * NXT : 3 * NXT] = BF16(1.0)
        in_maps.append({"xT": xT, "wxT": wxT, "whT": whT})
    return in_maps


def kernel(**inputs):
    embs = np.asarray(inputs["embs"], dtype=np.float32)
    depth = int(np.asarray(inputs["depth"]))
    assert depth == DEPTH and embs.shape == (2**DEPTH - 1, IN)
    Ws = {g + s: np.asarray(inputs["W" + g + s], dtype=np.float32) for g in GATES for s in "xh"}
    bs = {g: np.asarray(inputs["b" + g + "x"]) + np.asarray(inputs["b" + g + "h"]) for g in GATES}

    nc_a = _get_programs()
    in_maps = _prep_host_inputs(embs, Ws, bs)
    res = run_bass_kernel_spmd(nc_a, in_maps, core_ids=list(range(NCORE))).results

    root = res[0]["root"]  # [128, 16] f32: cols 0..7 = c M-tiles, 8..15 = h
    c_root = root[:, :8].T.reshape(MEM)
    h_root = root[:, 8:].T.reshape(MEM)
    return np.stack([c_root, h_root]).astype(np.float32)
